# revision 15
# baseline (speedup 1.0000x reference)
"""NonLocalAttention2D Trainium2 kernel.

Data-parallel over batch N=8: one image per NeuronCore.

Per-core math (x: (C=128, HW=4096) fp32):
  kv   = [Wv|Wk].T @ x                     (80, 4096)   PE
  pool = maxpool2x2(kv)                    (80, 1024)   ACT copy + DVE max
  A    = Wq @ k + Wq@bk                    (128, 1024)  PE + DVE bias-add
  scores_c(b) = A_c.T @ x_b                (128k, 512q) PE   -> psum
  attn = exp(scores)                       ACT psum->sbuf bf16 (no max-sub)
  av   = [v*e^{kbq} | e^{kbq}].T @ attn    (65, 512)    PE   row 64 = denominators
  r    = 1/denoms                          DVE reciprocal_approx_fast (1, 512)
  R65  = ones65.T @ r_bf16                 PE K=1 bf16 matmul broadcast -> psum
  aoTn = av_sb * R65                       DVE
  fin  = [g*Wo; g*(bo+bv@Wo)].T @ aoTn     (128, 512)   PE
  out  = fin + x_b                         DVE -> DMA out

All biases are folded host-side: bv into wfin's bias row (the aoTn ones-row
is exactly 1), bk into a per-output-channel A offset (Wq@bk) and into the
e^{k.bq} exponent bias (bk.bq).

Perf structure:
  - HAM clock gate: PE idle >3.4us re-throttles matmuls to 1.2GHz. A warmup
    matmul stream covers the input-DMA phase; the per-chunk prologue pipeline
    and software-pipelined main loop keep the PE duty cycle high after that.
  - The main loop is emitted software-pipelined: scores/exp of block b+1
    are enqueued BEFORE av/tail of block b, so the in-order PE queue never
    starves the ACT exp stream (the steady-state bottleneck, ~4.45us/block).
  - x loads as 8 x 256KB DMAs alternating between the two HWDGE rings;
    block 0's scores/exp are interleaved into the per-chunk prologue so
    compute overlaps the ~7us HBM-bound input load.
"""

import sys

if "/opt/trn_rl_repo" not in sys.path:
    sys.path.insert(0, "/opt/trn_rl_repo")

import numpy as np

import concourse.bacc as bacc
import concourse.bass as bass
import concourse.tile as tile
from concourse import bass_utils, masks, mybir

F32 = mybir.dt.float32
BF16 = mybir.dt.bfloat16
F32R = mybir.dt.float32r


C = 128          # channels
HW = 4096        # 64*64 pixels
L = 1024         # pooled keys (32*32)
D = 16           # attn dim
DV = 64          # value dim
QB = 512         # q-block size
NB = HW // QB    # 8 q blocks
KC = 128         # keys per chunk
NC_CHUNKS = L // KC  # 8 key chunks
NCORES = 8
N_WARM = 9       # upfront warmup matmuls (HAM needs ~3.4us sustained busy)

# packed weight tensor layout: [128, WPACK_W] fp32
#   cols 0:80    [Wv|Wk]            (rows 0:128)
#   cols 80:208  Wq.T               (rows 64:80, at k's partition base)
#   cols 208:336 [g*Wo; g*(bo+bv@Wo)] (rows 0:65)
#   col  336     Wq@bk              (rows 0:128)
#   col  337     bq                 (rows 64:80)
#   col  338     bk.bq              (rows 0:128, replicated scalar)
WPACK_W = 339


def build_kernel(variant="full"):
    nc = bacc.Bacc("TRN2", target_bir_lowering=False, debug=False)

    x_d = nc.dram_tensor("x", (C, HW), F32, kind="ExternalInput").ap()
    wp_d = nc.dram_tensor("wpack", (C, WPACK_W), F32, kind="ExternalInput").ap()
    out_d = nc.dram_tensor("out", (C, HW), F32, kind="ExternalOutput").ap()

    from contextlib import ExitStack

    with tile.TileContext(nc) as tc, ExitStack() as ctx:
        singles = ctx.enter_context(tc.tile_pool(name="singles", bufs=1))
        s1_pool = ctx.enter_context(tc.tile_pool(name="s1", bufs=2))
        attn_pool = ctx.enter_context(tc.tile_pool(name="attn", bufs=2))
        r_pool = ctx.enter_context(tc.tile_pool(name="r", bufs=2))
        rb_pool = ctx.enter_context(tc.tile_pool(name="rb", bufs=2))
        avs_pool = ctx.enter_context(tc.tile_pool(name="avs", bufs=2))
        ao_pool = ctx.enter_context(tc.tile_pool(name="ao", bufs=2))
        out_pool = ctx.enter_context(tc.tile_pool(name="outp", bufs=3))

        ps_score = ctx.enter_context(tc.tile_pool(name="ps_score", bufs=2, space="PSUM"))
        ps_av = ctx.enter_context(tc.tile_pool(name="ps_av", bufs=2, space="PSUM"))
        ps_fin = ctx.enter_context(tc.tile_pool(name="ps_fin", bufs=2, space="PSUM"))

        # ---- tiles ----
        wpack = singles.tile([C, WPACK_W], F32R, tag="wpack")
        ident = singles.tile([DV, DV], F32, tag="ident")
        ones65 = singles.tile([1, DV + 1], BF16, tag="ones65")
        warm = singles.tile([C, QB], BF16, tag="warm")

        x_sb = singles.tile([C, HW], F32R, tag="x")
        kv_pool = singles.tile([80, L], F32R, tag="kvp")
        a_sb = singles.tile([C, NC_CHUNKS * KC], F32R, tag="a")
        vaug_sb = singles.tile([KC, NC_CHUNKS * (DV + 1)], BF16, tag="vaug")
        ebqk_sb = singles.tile([KC, NC_CHUNKS], F32, tag="ebqk")

        w_kv = wpack[:, 0:80]
        w_qt = wpack[64:80, 80:208]
        w_fin = wpack[: DV + 1, 208:336]
        wq_bk = wpack[:, 336:337].bitcast(F32)
        b_q = wpack[64:80, 337:338].bitcast(F32)
        bkbq = wpack[:, 338:339].bitcast(F32)

        # warm-tile memset first on the DVE queue so the PE warmup stream
        # can start right at the framework start barrier
        nc.vector.memset(warm[:, :], 0.0)

        # ---- input DMAs (both rings start immediately) ----
        nc.scalar.dma_start(out=wpack, in_=wp_d.bitcast(F32R))
        for c in range(NB):
            sl = slice(c * QB, (c + 1) * QB)
            eng = nc.sync if c % 2 == 0 else nc.scalar
            eng.dma_start(out=x_sb[:, sl], in_=x_d[:, sl].bitcast(F32R))

        # ---- PE warmup stream: warms the HAM clock gate during the DMAs
        def emit_warm(n=1):
            for _ in range(n):
                wp = ps_fin.tile([C, QB], F32, tag="fin")
                nc.tensor.matmul(
                    wp[:, :], lhsT=warm[:, :KC], rhs=warm[:, :], start=True, stop=True
                )

        emit_warm(N_WARM)

        masks.make_identity(nc, ident[:, :])
        nc.vector.memset(ones65[:, :], 1.0)

        def emit_scores(b, c, sc):
            # one chunk of scores for block b into its psum tile slot
            j = c % 2
            qsl = slice(b * QB, (b + 1) * QB)
            nc.tensor.matmul(
                sc[:, j * QB : (j + 1) * QB],
                lhsT=a_sb[:, c * KC : (c + 1) * KC],
                rhs=x_sb[:, qsl],
                start=True,
                stop=True,
            )

        def emit_exp(b, t, sc, attn):
            nc.scalar.activation(
                attn[:, t * 2 * QB : (t + 1) * 2 * QB],
                sc[:, :],
                mybir.ActivationFunctionType.Exp,
            )

        # ---- chunk-pipelined prologue, with block 0's scores interleaved ----
        attn0 = attn_pool.tile([KC, NC_CHUNKS * QB], BF16, tag="attn")
        sc0 = None
        for c in range(NC_CHUNKS):
            sl = slice(c * QB, (c + 1) * QB)
            proj = ps_fin.tile([C, QB], F32, tag="fin")
            nc.tensor.matmul(
                proj[:80, :], lhsT=w_kv, rhs=x_sb[:, sl], start=True, stop=True
            )
            # maxpool2x2: ACT copies the even-w columns out of PSUM, DVE maxes
            # with the odd-w columns (TensorTensor: only one PSUM input), then
            # maxes the h-pairs into kv_pool (f32r - it feeds FP32r matmuls).
            pv = proj[:80, :].rearrange("p (w two) -> p w two", two=2)
            s1 = s1_pool.tile([80, 256], F32, tag="s1")
            nc.scalar.activation(
                s1[:, :], pv[:, :, 0], mybir.ActivationFunctionType.Copy
            )
            nc.vector.tensor_max(s1[:, :], s1[:, :], pv[:, :, 1])
            sv = s1.rearrange("p (h two w) -> p h two w", h=4, two=2)
            ov = kv_pool[:, c * KC : (c + 1) * KC].rearrange("p (h w) -> p h w", h=4)
            nc.vector.tensor_max(ov, sv[:, :, 0, :], sv[:, :, 1, :])

            # A_c = Wq @ k_c (+ Wq@bk as the copy's bias) straight away
            a_ps = ps_av.tile([C, KC], F32, tag="av")
            nc.tensor.matmul(
                a_ps[:, :],
                lhsT=w_qt,
                rhs=kv_pool[64:80, c * KC : (c + 1) * KC],
                start=True,
                stop=True,
            )
            nc.vector.tensor_scalar_add(
                a_sb[:, c * KC : (c + 1) * KC], a_ps[:, :], wq_bk
            )

            # block 0's scores for this chunk ride the same pipeline
            if c % 2 == 0:
                sc0 = ps_score.tile([KC, 2 * QB], F32, tag="sc")
            emit_scores(0, c, sc0)
            if c % 2 == 1:
                emit_exp(0, (c - 1) // 2, sc0, attn0)
            if c < 6:
                emit_warm(1)

        # bqk_c = k_c.T @ bq  (8 tiny fp32 matmuls into one psum tile), then
        # ebqk = exp(bqk + bk.bq)
        k_rows = kv_pool[64:80, :]
        bqk = ps_av.tile([KC, NC_CHUNKS], F32, tag="av")
        for c in range(NC_CHUNKS):
            ksl = slice(c * KC, (c + 1) * KC)
            nc.tensor.matmul(
                bqk[:, c : c + 1],
                lhsT=k_rows[:, ksl].bitcast(F32),
                rhs=b_q,
                start=(c == 0),
                stop=(c == NC_CHUNKS - 1),
                skip_group_check=True,
            )
        nc.scalar.activation(
            ebqk_sb[:, :], bqk[:, :], mybir.ActivationFunctionType.Exp, bias=bkbq
        )
        emit_warm(2)

        # vT chunks via PE transpose, scaled by e^{bqk}; col 64 = e^{bqk}
        for c in range(NC_CHUNKS):
            vt_ps = ps_av.tile([KC, DV], F32, tag="av")
            nc.tensor.transpose(
                vt_ps[:, :],
                kv_pool[:DV, c * KC : (c + 1) * KC].bitcast(F32),
                ident[:, :],
            )
            base = c * (DV + 1)
            nc.vector.tensor_scalar_mul(
                vaug_sb[:, base : base + DV], vt_ps[:, :], ebqk_sb[:, c : c + 1]
            )
            nc.vector.tensor_copy(
                vaug_sb[:, base + DV : base + DV + 1], ebqk_sb[:, c : c + 1]
            )

        # ---- software-pipelined main loop ----
        # stage order per iteration: scores+exp of block b, then av+tail of
        # block b-1 - so the PE queue never blocks the ACT exp cadence.
        attn_tiles = {0: attn0}

        def emit_block_scores(b):
            attn = attn_pool.tile([KC, NC_CHUNKS * QB], BF16, tag="attn")
            attn_tiles[b] = attn
            for t in range(4):
                sc = ps_score.tile([KC, 2 * QB], F32, tag="sc")
                emit_scores(b, 2 * t, sc)
                emit_scores(b, 2 * t + 1, sc)
                emit_exp(b, t, sc, attn)

        def emit_block_tail(b):
            qsl = slice(b * QB, (b + 1) * QB)
            attn = attn_tiles.pop(b)
            av = ps_av.tile([DV + 1, QB], F32, tag="av")
            for c in range(NC_CHUNKS):
                base = c * (DV + 1)
                nc.tensor.matmul(
                    av[:, :],
                    lhsT=vaug_sb[:, base : base + DV + 1],
                    rhs=attn[:, c * QB : (c + 1) * QB],
                    start=(c == 0),
                    stop=(c == NC_CHUNKS - 1),
                )
            # normalize: av_sb copy (f32r, also fin's rhs), fast reciprocal of
            # its denominator row, bf16 K=1 matmul broadcast, multiply.
            av_sb = avs_pool.tile([DV + 1, QB], F32R, tag="avs")
            nc.vector.tensor_copy(av_sb[:, :], av[:, :])
            # custom-DVE ops misread inputs at a nonzero partition base, so
            # stage the denominator row to a partition-0 tile first
            r0 = r_pool.tile([1, QB], F32, tag="r0")
            nc.vector.tensor_copy(r0[:, :], av[DV : DV + 1, :])
            r = r_pool.tile([1, QB], F32, tag="r")
            nc.vector.reciprocal_approx_fast(out=r[:, :], in_=r0[:, :])
            rb = rb_pool.tile([1, QB], BF16, tag="rb")
            nc.vector.tensor_copy(rb[:, :], r[:, :])
            R65p = ps_fin.tile([DV + 1, QB], F32, tag="fin")
            nc.tensor.matmul(
                R65p[:, :], lhsT=ones65[:, :], rhs=rb[:, :], start=True, stop=True
            )
            aoTn = ao_pool.tile([DV + 1, QB], F32R, tag="ao")
            nc.vector.tensor_mul(aoTn[:, :], av_sb[:, :], R65p[:, :])
            fin = ps_fin.tile([C, QB], F32, tag="fin")
            nc.tensor.matmul(
                fin[:, :], lhsT=w_fin, rhs=aoTn[:, :], start=True, stop=True
            )
            o_sb = out_pool.tile([C, QB], F32, tag="o")
            nc.vector.tensor_add(o_sb[:, :], fin[:, :], x_sb[:, qsl].bitcast(F32))
            nc.sync.dma_start(out=out_d[:, qsl], in_=o_sb[:, :])

        for b in range(1, NB):
            emit_block_scores(b)
            emit_block_tail(b - 1)
        emit_block_tail(NB - 1)

    nc.compile()
    return nc


def prep_weights(Wq, bq, Wk, bk, Wv, bv, Wo, bo, gamma):
    g = np.float32(np.asarray(gamma))
    Wq, bq = np.asarray(Wq), np.asarray(bq)
    Wk, bk = np.asarray(Wk), np.asarray(bk)
    Wv, bv = np.asarray(Wv), np.asarray(bv)
    Wo, bo = np.asarray(Wo), np.asarray(bo)
    wpack = np.zeros((C, WPACK_W), np.float32)
    wpack[:, 0:64] = Wv
    wpack[:, 64:80] = Wk
    wpack[64:80, 80:208] = Wq.T
    wpack[:DV, 208:336] = g * Wo
    wpack[DV, 208:336] = g * (bo + bv @ Wo)
    wpack[:, 336] = Wq @ bk
    wpack[64:80, 337] = bq
    wpack[:, 338] = np.float32(bk @ bq)
    return np.ascontiguousarray(wpack)


_NC_CACHE = {}


def kernel(x, Wq, bq, Wk, bk, Wv, bv, Wo, bo, gamma):
    x = np.asarray(x, dtype=np.float32)
    N = x.shape[0]
    assert x.shape == (N, C, 64, 64) and N == NCORES
    wpack = prep_weights(Wq, bq, Wk, bk, Wv, bv, Wo, bo, gamma)

    if "nc" not in _NC_CACHE:
        _NC_CACHE["nc"] = build_kernel()
    nc = _NC_CACHE["nc"]

    in_maps = []
    for i in range(N):
        in_maps.append(
            {
                "x": np.ascontiguousarray(x[i].reshape(C, HW)),
                "wpack": wpack,
            }
        )
    res = bass_utils.run_bass_kernel_spmd(nc, in_maps, core_ids=list(range(N)))
    out = np.stack([res.results[i]["out"].reshape(C, 64, 64) for i in range(N)])
    return out.astype(np.float32)


if __name__ == "__main__":
    rng = np.random.default_rng(0)
    x = rng.standard_normal((8, C, 64, 64), dtype=np.float32)
    print("built", build_kernel())


# revision 18
# speedup vs baseline: 1.1861x; 1.1861x over previous
"""NonLocalAttention2D Trainium2 kernel.

Data-parallel over batch N=8: one image per NeuronCore.

Per-core math (x: (C=128, HW=4096) fp32):
  kv   = [Wv|Wk].T @ x                     (80, 4096)   PE
  pool = maxpool2x2(kv)                    (80, 1024)   ACT copy + DVE max
  A    = Wq @ k + Wq@bk                    (128, 1024)  PE + DVE bias-add
  scores_c(b) = A_c.T @ x_b                (128k, 512q) PE   -> psum
  attn = exp(scores)                       ACT psum->sbuf bf16 (no max-sub)
  av   = [v*e^{kbq} | e^{kbq}].T @ attn    (65, 512)    PE   row 64 = denominators
  r    = 1/denoms                          DVE reciprocal_approx_fast (1, 512)
  R65  = ones65.T @ r_bf16                 PE K=1 bf16 matmul broadcast -> psum
  aoTn = av_sb * R65                       DVE
  fin  = [g*Wo; g*(bo+bv@Wo)].T @ aoTn     (128, 512)   PE
  out  = fin + x_b                         DVE -> DMA out

All biases are folded host-side: bv into wfin's bias row (the aoTn ones-row
is exactly 1), bk into a per-output-channel A offset (Wq@bk) and into the
e^{k.bq} exponent bias (bk.bq).

Perf structure:
  - HAM clock gate: PE idle >3.4us re-throttles matmuls to 1.2GHz. A warmup
    matmul stream covers the input-DMA phase; the per-chunk prologue pipeline
    and software-pipelined main loop keep the PE duty cycle high after that.
  - The main loop is emitted software-pipelined: scores/exp of block b+1
    are enqueued BEFORE av/tail of block b, so the in-order PE queue never
    starves the ACT exp stream (the steady-state bottleneck, ~4.45us/block).
  - x loads as 8 x 256KB DMAs alternating between the two HWDGE rings;
    block 0's scores/exp are interleaved into the per-chunk prologue so
    compute overlaps the ~7us HBM-bound input load.
"""

import sys

if "/opt/trn_rl_repo" not in sys.path:
    sys.path.insert(0, "/opt/trn_rl_repo")

import numpy as np

import concourse.bacc as bacc
import concourse.bass as bass
import concourse.tile as tile
from concourse import bass_utils, masks, mybir

F32 = mybir.dt.float32
BF16 = mybir.dt.bfloat16
F32R = mybir.dt.float32r


C = 128          # channels
HW = 4096        # 64*64 pixels
L = 1024         # pooled keys (32*32)
D = 16           # attn dim
DV = 64          # value dim
QB = 512         # q-block size
NB = HW // QB    # 8 q blocks
KC = 128         # keys per chunk
NC_CHUNKS = L // KC  # 8 key chunks
NCORES = 8
N_WARM = 9       # upfront warmup matmuls (HAM needs ~3.4us sustained busy)

# packed weight tensor layout: [128, WPACK_W] fp32
#   cols 0:80    [Wv|Wk]            (rows 0:128)
#   cols 80:208  Wq.T               (rows 64:80, at k's partition base)
#   cols 208:336 [g*Wo; g*(bo+bv@Wo)] (rows 0:65)
#   col  336     Wq@bk              (rows 0:128)
#   col  337     bq                 (rows 64:80)
#   col  338     bk.bq              (rows 0:128, replicated scalar)
WPACK_W = 339


def build_kernel(variant="full"):
    nc = bacc.Bacc("TRN2", target_bir_lowering=False, debug=False)

    x_d = nc.dram_tensor("x", (C, HW), F32, kind="ExternalInput").ap()
    wp_d = nc.dram_tensor("wpack", (C, WPACK_W), F32, kind="ExternalInput").ap()
    out_d = nc.dram_tensor("out", (C, HW), F32, kind="ExternalOutput").ap()

    from contextlib import ExitStack

    with tile.TileContext(nc) as tc, ExitStack() as ctx:
        singles = ctx.enter_context(tc.tile_pool(name="singles", bufs=1))
        s1_pool = ctx.enter_context(tc.tile_pool(name="s1", bufs=2))
        attn_pool = ctx.enter_context(tc.tile_pool(name="attn", bufs=2))
        r_pool = ctx.enter_context(tc.tile_pool(name="r", bufs=2))
        rb_pool = ctx.enter_context(tc.tile_pool(name="rb", bufs=2))
        avs_pool = ctx.enter_context(tc.tile_pool(name="avs", bufs=2))
        ao_pool = ctx.enter_context(tc.tile_pool(name="ao", bufs=2))
        out_pool = ctx.enter_context(tc.tile_pool(name="outp", bufs=3))

        ps_score = ctx.enter_context(tc.tile_pool(name="ps_score", bufs=2, space="PSUM"))
        ps_av = ctx.enter_context(tc.tile_pool(name="ps_av", bufs=2, space="PSUM"))
        ps_fin = ctx.enter_context(tc.tile_pool(name="ps_fin", bufs=2, space="PSUM"))

        # ---- tiles ----
        wpack = singles.tile([C, WPACK_W], F32R, tag="wpack")
        ident = singles.tile([DV, DV], F32, tag="ident")
        ones65 = singles.tile([1, DV + 1], BF16, tag="ones65")
        warm = singles.tile([C, QB], BF16, tag="warm")

        x_sb = singles.tile([C, HW], F32R, tag="x")
        kv_pool = singles.tile([80, L], F32R, tag="kvp")
        a_sb = singles.tile([C, NC_CHUNKS * KC], F32R, tag="a")
        vaug_sb = singles.tile([KC, NC_CHUNKS * (DV + 1)], BF16, tag="vaug")
        ebqk_sb = singles.tile([KC, NC_CHUNKS], F32, tag="ebqk")

        w_kv = wpack[:, 0:80]
        w_qt = wpack[64:80, 80:208]
        w_fin = wpack[: DV + 1, 208:336]
        wq_bk = wpack[:, 336:337].bitcast(F32)
        b_q = wpack[64:80, 337:338].bitcast(F32)
        bkbq = wpack[:, 338:339].bitcast(F32)

        # warm-tile memset first on the DVE queue so the PE warmup stream
        # can start right at the framework start barrier
        nc.vector.memset(warm[:, :], 0.0)

        # ---- input DMAs (both rings start immediately) ----
        nc.scalar.dma_start(out=wpack, in_=wp_d.bitcast(F32R))
        for c in range(NB):
            sl = slice(c * QB, (c + 1) * QB)
            eng = nc.sync if c % 2 == 0 else nc.scalar
            eng.dma_start(out=x_sb[:, sl], in_=x_d[:, sl].bitcast(F32R))

        # ---- PE warmup stream: warms the HAM clock gate during the DMAs
        def emit_warm(n=1):
            for _ in range(n):
                wp = ps_fin.tile([C, QB], F32, tag="fin")
                nc.tensor.matmul(
                    wp[:, :], lhsT=warm[:, :KC], rhs=warm[:, :], start=True, stop=True
                )

        emit_warm(N_WARM)

        masks.make_identity(nc, ident[:, :])
        nc.vector.memset(ones65[:, :], 1.0)

        def emit_scores(b, c, sc):
            # one chunk of scores for block b into its psum tile slot
            j = c % 2
            qsl = slice(b * QB, (b + 1) * QB)
            nc.tensor.matmul(
                sc[:, j * QB : (j + 1) * QB],
                lhsT=a_sb[:, c * KC : (c + 1) * KC],
                rhs=x_sb[:, qsl],
                start=True,
                stop=True,
            )

        def emit_exp(b, t, sc, attn):
            nc.scalar.activation(
                attn[:, t * 2 * QB : (t + 1) * 2 * QB],
                sc[:, :],
                mybir.ActivationFunctionType.Exp,
            )

        # ---- chunk-pipelined prologue, with block 0's scores interleaved ----
        attn0 = attn_pool.tile([KC, NC_CHUNKS * QB], BF16, tag="attn")
        sc0 = None
        for c in range(NC_CHUNKS):
            sl = slice(c * QB, (c + 1) * QB)
            proj = ps_fin.tile([C, QB], F32, tag="fin")
            nc.tensor.matmul(
                proj[:80, :], lhsT=w_kv, rhs=x_sb[:, sl], start=True, stop=True
            )
            # maxpool2x2: ACT copies the even-w columns out of PSUM, DVE maxes
            # with the odd-w columns (TensorTensor: only one PSUM input), then
            # maxes the h-pairs into kv_pool (f32r - it feeds FP32r matmuls).
            pv = proj[:80, :].rearrange("p (w two) -> p w two", two=2)
            s1 = s1_pool.tile([80, 256], F32, tag="s1")
            nc.scalar.activation(
                s1[:, :], pv[:, :, 0], mybir.ActivationFunctionType.Copy
            )
            nc.vector.tensor_max(s1[:, :], s1[:, :], pv[:, :, 1])
            sv = s1.rearrange("p (h two w) -> p h two w", h=4, two=2)
            ov = kv_pool[:, c * KC : (c + 1) * KC].rearrange("p (h w) -> p h w", h=4)
            nc.vector.tensor_max(ov, sv[:, :, 0, :], sv[:, :, 1, :])

            # A_c = Wq @ k_c (+ Wq@bk as the copy's bias) straight away
            a_ps = ps_av.tile([C, KC], F32, tag="av")
            nc.tensor.matmul(
                a_ps[:, :],
                lhsT=w_qt,
                rhs=kv_pool[64:80, c * KC : (c + 1) * KC],
                start=True,
                stop=True,
            )
            nc.vector.tensor_scalar_add(
                a_sb[:, c * KC : (c + 1) * KC], a_ps[:, :], wq_bk
            )

            # block 0's scores for this chunk ride the same pipeline
            if c % 2 == 0:
                sc0 = ps_score.tile([KC, 2 * QB], F32, tag="sc")
            emit_scores(0, c, sc0)
            if c % 2 == 1:
                emit_exp(0, (c - 1) // 2, sc0, attn0)
            if c < 6:
                emit_warm(1)

        # bqk_c = k_c.T @ bq  (8 tiny fp32 matmuls into one psum tile), then
        # ebqk = exp(bqk + bk.bq)
        k_rows = kv_pool[64:80, :]
        bqk = ps_av.tile([KC, NC_CHUNKS], F32, tag="av")
        for c in range(NC_CHUNKS):
            ksl = slice(c * KC, (c + 1) * KC)
            nc.tensor.matmul(
                bqk[:, c : c + 1],
                lhsT=k_rows[:, ksl].bitcast(F32),
                rhs=b_q,
                start=(c == 0),
                stop=(c == NC_CHUNKS - 1),
                skip_group_check=True,
            )
        nc.scalar.activation(
            ebqk_sb[:, :], bqk[:, :], mybir.ActivationFunctionType.Exp, bias=bkbq
        )
        emit_warm(2)

        # vT chunks via PE transpose, scaled by e^{bqk}; col 64 = e^{bqk}
        for c in range(NC_CHUNKS):
            vt_ps = ps_av.tile([KC, DV], F32, tag="av")
            nc.tensor.transpose(
                vt_ps[:, :],
                kv_pool[:DV, c * KC : (c + 1) * KC].bitcast(F32),
                ident[:, :],
            )
            base = c * (DV + 1)
            nc.vector.tensor_scalar_mul(
                vaug_sb[:, base : base + DV], vt_ps[:, :], ebqk_sb[:, c : c + 1]
            )
            nc.vector.tensor_copy(
                vaug_sb[:, base + DV : base + DV + 1], ebqk_sb[:, c : c + 1]
            )

        # ---- software-pipelined main loop ----
        # stage order per iteration: scores+exp of block b, then av+tail of
        # block b-1 - so the PE queue never blocks the ACT exp cadence.
        attn_tiles = {0: attn0}

        def emit_block_scores(b):
            attn = attn_pool.tile([KC, NC_CHUNKS * QB], BF16, tag="attn")
            attn_tiles[b] = attn
            for t in range(4):
                sc = ps_score.tile([KC, 2 * QB], F32, tag="sc")
                emit_scores(b, 2 * t, sc)
                emit_scores(b, 2 * t + 1, sc)
                emit_exp(b, t, sc, attn)

        def emit_block_tail(b):
            qsl = slice(b * QB, (b + 1) * QB)
            attn = attn_tiles.pop(b)
            av = ps_av.tile([DV + 1, QB], F32, tag="av")
            for c in range(NC_CHUNKS):
                base = c * (DV + 1)
                nc.tensor.matmul(
                    av[:, :],
                    lhsT=vaug_sb[:, base : base + DV + 1],
                    rhs=attn[:, c * QB : (c + 1) * QB],
                    start=(c == 0),
                    stop=(c == NC_CHUNKS - 1),
                )
            # normalize: av_sb copy (f32r, also fin's rhs), fast reciprocal of
            # its denominator row, bf16 K=1 matmul broadcast, multiply.
            av_sb = avs_pool.tile([DV + 1, QB], F32R, tag="avs")
            nc.vector.tensor_copy(av_sb[:, :], av[:, :])
            # custom-DVE ops misread inputs at a nonzero partition base, so
            # stage the denominator row to a partition-0 tile first
            r0 = r_pool.tile([1, QB], F32, tag="r0")
            nc.vector.tensor_copy(r0[:, :], av[DV : DV + 1, :])
            r = r_pool.tile([1, QB], F32, tag="r")
            nc.vector.reciprocal_approx_fast(out=r[:, :], in_=r0[:, :])
            rb = rb_pool.tile([1, QB], BF16, tag="rb")
            nc.vector.tensor_copy(rb[:, :], r[:, :])
            R65p = ps_fin.tile([DV + 1, QB], F32, tag="fin")
            nc.tensor.matmul(
                R65p[:, :], lhsT=ones65[:, :], rhs=rb[:, :], start=True, stop=True
            )
            aoTn = ao_pool.tile([DV + 1, QB], F32R, tag="ao")
            nc.vector.tensor_mul(aoTn[:, :], av_sb[:, :], R65p[:, :])
            fin = ps_fin.tile([C, QB], F32, tag="fin")
            nc.tensor.matmul(
                fin[:, :], lhsT=w_fin, rhs=aoTn[:, :], start=True, stop=True
            )
            o_sb = out_pool.tile([C, QB], F32, tag="o")
            nc.vector.tensor_add(o_sb[:, :], fin[:, :], x_sb[:, qsl].bitcast(F32))
            nc.sync.dma_start(out=out_d[:, qsl], in_=o_sb[:, :])

        for b in range(1, NB):
            emit_block_scores(b)
            emit_block_tail(b - 1)
        emit_block_tail(NB - 1)

    nc.compile()
    return nc


def prep_weights(Wq, bq, Wk, bk, Wv, bv, Wo, bo, gamma):
    g = np.float32(np.asarray(gamma))
    Wq, bq = np.asarray(Wq), np.asarray(bq)
    Wk, bk = np.asarray(Wk), np.asarray(bk)
    Wv, bv = np.asarray(Wv), np.asarray(bv)
    Wo, bo = np.asarray(Wo), np.asarray(bo)
    wpack = np.zeros((C, WPACK_W), np.float32)
    wpack[:, 0:64] = Wv
    wpack[:, 64:80] = Wk
    wpack[64:80, 80:208] = Wq.T
    wpack[:DV, 208:336] = g * Wo
    wpack[DV, 208:336] = g * (bo + bv @ Wo)
    wpack[:, 336] = Wq @ bk
    wpack[64:80, 337] = bq
    wpack[:, 338] = np.float32(bk @ bq)
    return np.ascontiguousarray(wpack)


_NC_CACHE = {}


def kernel(x, Wq, bq, Wk, bk, Wv, bv, Wo, bo, gamma):
    x = np.asarray(x, dtype=np.float32)
    N = x.shape[0]
    assert x.shape == (N, C, 64, 64) and N == NCORES
    wpack = prep_weights(Wq, bq, Wk, bk, Wv, bv, Wo, bo, gamma)

    if "nc" not in _NC_CACHE:
        _NC_CACHE["nc"] = build_kernel()
    nc = _NC_CACHE["nc"]

    in_maps = []
    for i in range(N):
        in_maps.append(
            {
                "x": np.ascontiguousarray(x[i].reshape(C, HW)),
                "wpack": wpack,
            }
        )
    res = bass_utils.run_bass_kernel_spmd(nc, in_maps, core_ids=list(range(N)))
    out = np.stack([res.results[i]["out"].reshape(C, 64, 64) for i in range(N)])
    return out.astype(np.float32)


if __name__ == "__main__":
    rng = np.random.default_rng(0)
    x = rng.standard_normal((8, C, 64, 64), dtype=np.float32)
    print("built", build_kernel())
